# revision 7
# baseline (speedup 1.0000x reference)
"""GNN message-passing layer (EquivariantMPLayer) on 8 Trainium2 NeuronCores.

Sharding: edges are sharded by destination-node range (dst // (N/8)) so each
core aggregates its own node range locally -- no collectives needed. Per core,
edges are sorted by dst and grouped into 128-node windows; each window's edge
list is padded to 128-edge blocks. Per-window block counts are equalized
across cores (max over cores) so a single SPMD program serves all 8 cores.

The host pre-gathers x[src] and x[dst] for every edge slot into a single
feature-major stream xcatT [128, epad] (rows 0:64 = src feats, 64:128 = dst
feats, columns in device consumption order), so the device needs no gathers
and no transposes: it just streams sequential DMA.

The device pipeline is software-pipelined so the tensor engine never waits on
the scalar/vector engines: at iteration k it runs L1(k), L2(k-1) and
scatter(k-2), while ACT runs silu(k)/msg-copy(k-1) and DVE builds the
one-hots (one batched is_equal per 512-edge group) and applies the
per-window mean/bias flush. The update MLP + LayerNorm groups are emitted as
soon as their 4-window input range has been flushed, overlapping the node
phase with the tail of the edge phase.
"""

import numpy as np

N = 50000
E = 800000
DIN = 64
DOUT = 64
NB = 16
MAX_RADIUS = 10.0
NCORES = 8
P = 128

_prog_cache = {}


# ---------------------------------------------------------------------------
# Host-side structure / metadata
# ---------------------------------------------------------------------------

def _build_host_data(x, edge_index, edge_len, mw1, mb1, mw2, mb2,
                     uw1, ub1, uw2, ub2, ln_g, ln_b,
                     n=N, ncores=NCORES):
    import ml_dtypes
    bf16 = ml_dtypes.bfloat16

    nloc = n // ncores
    nw = (nloc + P - 1) // P
    npad = nw * P

    src = np.asarray(edge_index[0], dtype=np.int64)
    dst = np.asarray(edge_index[1], dtype=np.int64)
    x = np.asarray(x, dtype=np.float32)
    el = np.asarray(edge_len, dtype=np.float32)[:, 0]

    centers = np.linspace(0.0, MAX_RADIUS, NB, dtype=np.float64)
    width = (centers[1] - centers[0]) * 0.5
    rbf_all = np.exp(-((el[:, None].astype(np.float64) - centers) ** 2)
                     / (2.0 * width ** 2)).astype(np.float32)  # [E, 16]

    core_of = dst // nloc
    per_core = []
    cnt_w = np.zeros((ncores, nw), dtype=np.int64)
    for c in range(ncores):
        eids = np.nonzero(core_of == c)[0]
        dloc = (dst[eids] - c * nloc).astype(np.int64)
        order = np.argsort(dloc, kind="stable")
        eids = eids[order]
        dloc = dloc[order]
        w_of = dloc // P
        cnt_w[c] = np.bincount(w_of, minlength=nw)
        per_core.append((eids, dloc, w_of))

    # per-window block counts, equalized across cores; total padded to x4
    bws = np.maximum(1, (cnt_w.max(axis=0) + P - 1) // P)  # [nw]
    bws[-1] += (-int(bws.sum())) % 4
    btot = int(bws.sum())
    epad = btot * P

    block_window = np.repeat(np.arange(nw), bws)
    boff = np.concatenate([[0], np.cumsum(bws)[:-1]])  # first block of window

    in_maps = []
    for c in range(ncores):
        eids, dloc, w_of = per_core[c]
        # slot index for each edge: window base + position within window
        win_start = np.concatenate([[0], np.cumsum(cnt_w[c])[:-1]])
        pos_in_w = np.arange(len(eids)) - win_start[w_of]
        slot = boff[w_of] * P + pos_in_w  # [e_c]

        xcat = np.zeros((epad, 2 * DIN), dtype=np.float32)
        xcat[slot, :DIN] = x[src[eids]]
        xcat[slot, DIN:] = x[dst[eids]]
        xcatT = np.ascontiguousarray(xcat.T).astype(bf16)

        rbf = np.zeros((epad, NB), dtype=np.float32)
        rbf[slot] = rbf_all[eids]
        rbfT = np.ascontiguousarray(rbf.T).astype(bf16)

        dwrel = np.full(epad, 999.0, dtype=np.float32)
        dwrel[slot] = (dloc - w_of * P).astype(np.float32)
        dwrelT = np.ascontiguousarray(
            dwrel.reshape(btot, P).T).astype(bf16)  # [128, btot]

        cnt_n = np.zeros(npad, dtype=np.float32)
        cnt_n[:nloc] = np.bincount(dloc, minlength=nloc).astype(np.float32)
        inv = 1.0 / np.maximum(cnt_n, 1.0)
        has = (cnt_n > 0).astype(np.float32)
        fmul = np.broadcast_to(inv[None, :], (DOUT, npad)).copy()
        fadd = (np.asarray(mb2, np.float32)[:, None] * has[None, :]).copy()

        xt_loc = np.zeros((DIN, npad), dtype=np.float32)
        xt_loc[:, :nloc] = x[c * nloc:(c + 1) * nloc].T

        m = {
            "xcatT": xcatT,
            "rbfT": rbfT,
            "dwrelT": dwrelT,
            "xTloc": xt_loc,
            "fmul": fmul,
            "fadd": fadd,
            "mw1_sd": np.asarray(mw1, np.float32)[:2 * DIN].astype(bf16),
            "mw1_r": np.asarray(mw1, np.float32)[2 * DIN:].astype(bf16),
            "mb1": np.asarray(mb1, np.float32).reshape(2 * DOUT, 1).copy(),
            "mw2": np.asarray(mw2, np.float32).astype(bf16),
            # upd layout is [agg; x] -> swap uw1 row blocks to match
            "uw1": np.concatenate([np.asarray(uw1, np.float32)[DIN:],
                                   np.asarray(uw1, np.float32)[:DIN]], axis=0),
            "ub1": np.asarray(ub1, np.float32).reshape(DOUT, 1).copy(),
            "uw2": np.asarray(uw2, np.float32),
            "ub2": np.asarray(ub2, np.float32).reshape(DOUT, 1).copy(),
            "lng": np.broadcast_to(np.asarray(ln_g, np.float32)[None, :],
                                   (P, DOUT)).copy(),
            "lnb": np.broadcast_to(np.asarray(ln_b, np.float32)[None, :],
                                   (P, DOUT)).copy(),
            "iota4": np.tile(np.arange(P, dtype=np.float32).astype(bf16),
                             (P, 4)).copy(),
            "identf": np.eye(P, dtype=np.float32),
        }
        in_maps.append(m)

    struct = dict(n=n, nloc=nloc, nw=nw, npad=npad, btot=btot, epad=epad,
                  bws=tuple(int(v) for v in bws),
                  block_window=tuple(int(v) for v in block_window))
    return struct, in_maps


# ---------------------------------------------------------------------------
# Device program
# ---------------------------------------------------------------------------

def _build_program(struct):
    import concourse.bass as bass
    import concourse.mybir as mybir
    import concourse.tile as tile
    from concourse import bacc

    f32 = mybir.dt.float32
    bf = mybir.dt.bfloat16
    n, nloc, nw, npad = (struct["n"], struct["nloc"], struct["nw"],
                         struct["npad"])
    btot, epad = struct["btot"], struct["epad"]
    block_window = struct["block_window"]
    ngrp = btot // 4

    # first/last block of each window
    wfirst = {}
    wlast = {}
    for g, w in enumerate(block_window):
        wfirst.setdefault(w, g)
        wlast[w] = g

    nc = bacc.Bacc("TRN2", target_bir_lowering=False, debug=False,
                   enable_asserts=False, num_devices=NCORES)

    xcatT_d = nc.dram_tensor("xcatT", [P, epad], bf, kind="ExternalInput")
    rbfT_d = nc.dram_tensor("rbfT", [NB, epad], bf, kind="ExternalInput")
    dwrelT_d = nc.dram_tensor("dwrelT", [P, btot], bf, kind="ExternalInput")
    xTloc_d = nc.dram_tensor("xTloc", [DIN, npad], f32, kind="ExternalInput")
    fmul_d = nc.dram_tensor("fmul", [DOUT, npad], f32, kind="ExternalInput")
    fadd_d = nc.dram_tensor("fadd", [DOUT, npad], f32, kind="ExternalInput")
    mw1_sd_d = nc.dram_tensor("mw1_sd", [2 * DIN, 2 * DOUT], bf,
                              kind="ExternalInput")
    mw1_r_d = nc.dram_tensor("mw1_r", [NB, 2 * DOUT], bf,
                             kind="ExternalInput")
    mb1_d = nc.dram_tensor("mb1", [2 * DOUT, 1], f32, kind="ExternalInput")
    mw2_d = nc.dram_tensor("mw2", [2 * DOUT, DOUT], bf, kind="ExternalInput")
    uw1_d = nc.dram_tensor("uw1", [DIN + DOUT, DOUT], f32,
                           kind="ExternalInput")
    ub1_d = nc.dram_tensor("ub1", [DOUT, 1], f32, kind="ExternalInput")
    uw2_d = nc.dram_tensor("uw2", [DOUT, DOUT], f32, kind="ExternalInput")
    ub2_d = nc.dram_tensor("ub2", [DOUT, 1], f32, kind="ExternalInput")
    lng_d = nc.dram_tensor("lng", [P, DOUT], f32, kind="ExternalInput")
    lnb_d = nc.dram_tensor("lnb", [P, DOUT], f32, kind="ExternalInput")
    iota4_d = nc.dram_tensor("iota4", [P, 4 * P], bf, kind="ExternalInput")
    identf_d = nc.dram_tensor("identf", [P, P], f32, kind="ExternalInput")
    out_d = nc.dram_tensor("out", [npad, DOUT], f32, kind="ExternalOutput")

    AX = mybir.AxisListType
    OP = mybir.AluOpType
    ACT = mybir.ActivationFunctionType

    with tile.TileContext(nc) as tc:
        with (
            tc.tile_pool(name="const", bufs=1) as cpool,
            tc.tile_pool(name="gath", bufs=5) as gpool,
            tc.tile_pool(name="work", bufs=4) as wpool,
            tc.tile_pool(name="oh", bufs=3) as opool,
            tc.tile_pool(name="pt", bufs=1, space="PSUM") as pt_pool,
            tc.tile_pool(name="ph", bufs=3, space="PSUM") as ph_pool,
            tc.tile_pool(name="pm", bufs=2, space="PSUM") as pm_pool,
            tc.tile_pool(name="pa", bufs=2, space="PSUM") as pa_pool,
        ):
            def cload(dram, shape, dtype=f32):
                t = cpool.tile(shape, dtype, name=dram.name + "_t")
                nc.sync.dma_start(out=t[:], in_=dram[:])
                return t

            mw1_sd_t = cload(mw1_sd_d, [2 * DIN, 2 * DOUT], bf)
            mw1_r_t = cload(mw1_r_d, [NB, 2 * DOUT], bf)
            mb1_t = cload(mb1_d, [2 * DOUT, 1])
            mw2_t = cload(mw2_d, [2 * DOUT, DOUT], bf)
            uw1_t = cload(uw1_d, [DIN + DOUT, DOUT])
            ub1_t = cload(ub1_d, [DOUT, 1])
            uw2_t = cload(uw2_d, [DOUT, DOUT])
            ub2_t = cload(ub2_d, [DOUT, 1])
            lng_t = cload(lng_d, [P, DOUT])
            lnb_t = cload(lnb_d, [P, DOUT])
            iota4_t = cload(iota4_d, [P, 4 * P], bf)
            identf_t = cload(identf_d, [P, P])
            dwrelT_t = cload(dwrelT_d, [P, btot], bf)
            fmul_t = cload(fmul_d, [DOUT, npad])
            fadd_t = cload(fadd_d, [DOUT, npad])

            eps_t = cpool.tile([P, 1], f32, name="eps_t")
            nc.vector.memset(eps_t[:], 1e-5)

            # combined update-MLP input: rows 0:64 = aggT, rows 64:128 = xT
            upd_t = cpool.tile([P, npad], f32, name="upd_t")
            nc.sync.dma_start(out=upd_t[DOUT:P, :], in_=xTloc_d[:])

            # pipeline state
            st = {}     # k -> dict(xc, rb, ph, hT, pm, msg, oh4)
            pa_cur = {}
            flushed_w = [-1]
            next_u = [0]
            nug = (npad + 511) // 512

            def stage_l1(k):
                e0 = k * 4 * P
                s = st[k] = {}
                xc = s["xc"] = gpool.tile([P, 4 * P], bf, tag="xc",
                                          name=f"xc_{k}")
                nc.sync.dma_start(out=xc[:], in_=xcatT_d[:, e0:e0 + 4 * P])
                rb = s["rb"] = gpool.tile([NB, 4 * P], bf, tag="rb",
                                          name=f"rb_{k}")
                nc.sync.dma_start(out=rb[:], in_=rbfT_d[:, e0:e0 + 4 * P])
                ph = s["ph"] = ph_pool.tile([P, 4 * P], f32, tag="ph",
                                            name=f"ph_{k}")
                nc.tensor.matmul(ph[:], mw1_sd_t[:], xc[:],
                                 start=True, stop=False)
                nc.tensor.matmul(ph[:], mw1_r_t[:], rb[:],
                                 start=False, stop=True)
                hT = s["hT"] = wpool.tile([P, 4 * P], bf, tag="hT",
                                          name=f"hT_{k}")
                nc.scalar.activation(out=hT[:], in_=ph[:],
                                     func=ACT.Silu, bias=mb1_t[:, 0:1])
                # batched one-hot for the 4 blocks of this group
                oh4 = s["oh4"] = opool.tile([P, 4 * P], bf, tag="oh4",
                                            name=f"oh4_{k}")
                o3 = oh4[:].rearrange("p (j n) -> p j n", n=P)
                i3 = iota4_t[:].rearrange("p (j n) -> p j n", n=P)
                nc.vector.tensor_tensor(
                    out=o3, in0=i3,
                    in1=dwrelT_t[:, 4 * k:4 * k + 4, None]
                        .to_broadcast([P, 4, P]),
                    op=OP.is_equal)

            def stage_l2(k):
                s = st[k]
                hT = s["hT"]
                pm = s["pm"] = pm_pool.tile([P, 4 * DOUT], f32, tag="pm",
                                            name=f"pm_{k}")
                for j in range(4):
                    nc.tensor.matmul(pm[:, j * DOUT:(j + 1) * DOUT],
                                     hT[:, j * P:(j + 1) * P],
                                     mw2_t[:], start=True, stop=True)
                msg = s["msg"] = wpool.tile([P, 4 * DOUT], bf, tag="msg",
                                            name=f"msg_{k}")
                nc.scalar.copy(out=msg[:], in_=pm[:])

            def stage_scatter(k):
                s = st[k]
                msg, oh4 = s["msg"], s["oh4"]
                for j in range(4):
                    g = 4 * k + j
                    w = block_window[g]
                    if g == wfirst[w]:
                        pa_cur[w] = pa_pool.tile([DOUT, P], f32, tag="pa",
                                                 name=f"pa_w{w}")
                    nc.tensor.matmul(
                        pa_cur[w][:],
                        msg[:, j * DOUT:(j + 1) * DOUT],
                        oh4[:, j * P:(j + 1) * P],
                        start=(g == wfirst[w]),
                        stop=(g == wlast[w]),
                        skip_group_check=True)
                    if g != wlast[w]:
                        continue
                    wc = slice(w * P, (w + 1) * P)
                    nc.vector.tensor_tensor(
                        out=upd_t[0:DOUT, wc], in0=pa_cur[w][:],
                        in1=fmul_t[:, wc], op=OP.mult)
                    nc.vector.tensor_tensor(
                        out=upd_t[0:DOUT, wc], in0=upd_t[0:DOUT, wc],
                        in1=fadd_t[:, wc], op=OP.add)
                    del pa_cur[w]
                    flushed_w[0] = w
                del st[k]

            def stage_update(m):
                u0 = m * 512
                cw = min(512, npad - u0)
                nj = cw // P
                pu = ph_pool.tile([P, 512], f32, tag="ph", name=f"pu_{m}")
                nc.tensor.matmul(pu[0:DOUT, 0:cw], uw1_t[:],
                                 upd_t[:, u0:u0 + cw], start=True, stop=True)
                uh_sb = wpool.tile([DOUT, 512], f32, tag="uh", name=f"uh_{m}")
                nc.scalar.activation(out=uh_sb[:, 0:cw],
                                     in_=pu[0:DOUT, 0:cw],
                                     func=ACT.Silu, bias=ub1_t[:, 0:1])
                pz = pt_pool.tile([P, 512], f32, tag="pz", name=f"pz_{m}")
                nc.tensor.matmul(pz[0:DOUT, 0:cw], uw2_t[:], uh_sb[:, 0:cw],
                                 start=True, stop=True)
                zT_sb = wpool.tile([DOUT, 512], f32, tag="zT", name=f"zT_{m}")
                nc.scalar.activation(out=zT_sb[:, 0:cw], in_=pz[0:DOUT, 0:cw],
                                     func=ACT.Identity, bias=ub2_t[:, 0:1])

                pz2 = pm_pool.tile([P, 4 * DOUT], f32, tag="pm",
                                   name=f"pz2_{m}")
                for j in range(nj):
                    nc.tensor.transpose(
                        out=pz2[:, j * DOUT:(j + 1) * DOUT],
                        in_=zT_sb[:, j * P:(j + 1) * P],
                        identity=identf_t[0:DOUT, 0:DOUT])
                # LayerNorm on [128, nj, 64] (free-axis per-node)
                zc = wpool.tile([P, 4 * DOUT], f32, tag="zc", name=f"zc_{m}")
                red = wpool.tile([P, 4], f32, tag="red", name=f"red_{m}")
                red2 = wpool.tile([P, 4], f32, tag="red2", name=f"red2_{m}")
                z3 = pz2[:, 0:nj * DOUT].rearrange("p (j d) -> p j d", d=DOUT)
                nc.vector.tensor_reduce(out=red[:, 0:nj], in_=z3, axis=AX.X,
                                        op=OP.add)
                nc.vector.tensor_scalar_mul(red[:, 0:nj], red[:, 0:nj],
                                            -1.0 / DOUT)
                zc3 = zc[:, 0:nj * DOUT].rearrange("p (j d) -> p j d", d=DOUT)
                nc.vector.tensor_tensor(
                    out=zc3, in0=z3,
                    in1=red[:, 0:nj, None].to_broadcast([P, nj, DOUT]),
                    op=OP.add)
                sq = wpool.tile([P, 4 * DOUT], f32, tag="sq", name=f"sq_{m}")
                sq3 = sq[:, 0:nj * DOUT].rearrange("p (j d) -> p j d", d=DOUT)
                nc.vector.tensor_tensor(out=sq3, in0=zc3, in1=zc3, op=OP.mult)
                nc.vector.tensor_reduce(out=red2[:, 0:nj], in_=sq3, axis=AX.X,
                                        op=OP.add)
                sd = wpool.tile([P, 4], f32, tag="sd", name=f"sd_{m}")
                nc.scalar.activation(out=sd[:, 0:nj], in_=red2[:, 0:nj],
                                     func=ACT.Sqrt, scale=1.0 / DOUT,
                                     bias=eps_t[:, 0:1])
                rs = wpool.tile([P, 4], f32, tag="rs", name=f"rs_{m}")
                nc.vector.reciprocal(out=rs[:, 0:nj], in_=sd[:, 0:nj])
                zn = wpool.tile([P, 4 * DOUT], f32, tag="zn", name=f"zn_{m}")
                zn3 = zn[:, 0:nj * DOUT].rearrange("p (j d) -> p j d", d=DOUT)
                nc.vector.tensor_tensor(
                    out=zn3, in0=zc3,
                    in1=rs[:, 0:nj, None].to_broadcast([P, nj, DOUT]),
                    op=OP.mult)
                for j in range(nj):
                    js = slice(j * DOUT, (j + 1) * DOUT)
                    nc.vector.tensor_tensor(out=zn[:, js], in0=zn[:, js],
                                            in1=lng_t[:], op=OP.mult)
                    nc.vector.tensor_tensor(out=zn[:, js], in0=zn[:, js],
                                            in1=lnb_t[:], op=OP.add)
                    r0 = u0 + j * P
                    nc.sync.dma_start(out=out_d[r0:r0 + P, :],
                                      in_=zn[:, js])

            for k in range(ngrp + 2):
                if k < ngrp:
                    stage_l1(k)
                if 1 <= k <= ngrp:
                    stage_l2(k - 1)
                if k >= 2:
                    stage_scatter(k - 2)
                    while (next_u[0] < nug
                           and flushed_w[0] >= min(4 * next_u[0] + 3,
                                                   nw - 1)):
                        stage_update(next_u[0])
                        next_u[0] += 1
            while next_u[0] < nug:
                stage_update(next_u[0])
                next_u[0] += 1

    nc.compile()
    return nc


# ---------------------------------------------------------------------------
# Entry point
# ---------------------------------------------------------------------------

last_results = None


def kernel(x, edge_index, edge_vec, edge_len,
           mw1, mb1, mw2, mb2, uw1, ub1, uw2, ub2, ln_g, ln_b):
    global last_results
    import os
    from concourse.bass_utils import run_bass_kernel_spmd

    struct, in_maps = _build_host_data(
        x, edge_index, edge_len, mw1, mb1, mw2, mb2,
        uw1, ub1, uw2, ub2, ln_g, ln_b)

    key = (struct["n"], struct["btot"], struct["bws"])
    if key not in _prog_cache:
        _prog_cache[key] = _build_program(struct)
    nc = _prog_cache[key]

    kw = {}
    if os.environ.get("K_TRACE", ""):
        import profile_shim
        profile_shim.install()
        kw = dict(trace=True, trace_cores=list(range(NCORES)),
                  tmpdir="/tmp/ntff_out")
    res = run_bass_kernel_spmd(nc, in_maps, core_ids=list(range(NCORES)), **kw)
    last_results = res
    nloc = struct["nloc"]
    out = np.concatenate([res.results[c]["out"][:nloc] for c in range(NCORES)],
                         axis=0)
    return out.astype(np.float32)


# revision 17
# speedup vs baseline: 1.0082x; 1.0082x over previous
"""GNN message-passing layer (EquivariantMPLayer) on 8 Trainium2 NeuronCores.

Sharding: edges are sharded by destination-node range (dst // (N/8)) so each
core aggregates its own node range locally -- no collectives needed. Per core,
edges are sorted by dst and grouped into 128-node windows; each window's edge
list is padded to 128-edge blocks. Per-window block counts are equalized
across cores (max over cores) so a single SPMD program serves all 8 cores.

The host pre-gathers x[src] and x[dst] for every edge slot into a single
feature-major stream xcatT [128, epad] (rows 0:64 = src feats, 64:128 = dst
feats, columns in device consumption order), so the device needs no gathers
and no transposes: it just streams sequential DMA.

The device pipeline is software-pipelined so the tensor engine never waits on
the scalar/vector engines: at iteration k it runs L1(k), L2(k-1) and
scatter(k-2), while ACT runs silu(k)/msg-copy(k-1) and DVE builds the
one-hots (one batched is_equal per 512-edge group) and applies the
per-window mean/bias flush. The update MLP + LayerNorm groups are emitted as
soon as their 4-window input range has been flushed, overlapping the node
phase with the tail of the edge phase.
"""

import numpy as np

N = 50000
E = 800000
DIN = 64
DOUT = 64
NB = 16
MAX_RADIUS = 10.0
NCORES = 8
P = 128

_prog_cache = {}


# ---------------------------------------------------------------------------
# Host-side structure / metadata
# ---------------------------------------------------------------------------

def _build_host_data(x, edge_index, edge_len, mw1, mb1, mw2, mb2,
                     uw1, ub1, uw2, ub2, ln_g, ln_b,
                     n=N, ncores=NCORES):
    import ml_dtypes
    bf16 = ml_dtypes.bfloat16

    nloc = n // ncores
    nw = (nloc + P - 1) // P
    npad = nw * P

    src = np.asarray(edge_index[0], dtype=np.int64)
    dst = np.asarray(edge_index[1], dtype=np.int64)
    x = np.asarray(x, dtype=np.float32)
    el = np.asarray(edge_len, dtype=np.float32)[:, 0]

    centers = np.linspace(0.0, MAX_RADIUS, NB, dtype=np.float64)
    width = (centers[1] - centers[0]) * 0.5
    rbf_all = np.exp(-((el[:, None].astype(np.float64) - centers) ** 2)
                     / (2.0 * width ** 2)).astype(np.float32)  # [E, 16]

    core_of = dst // nloc
    per_core = []
    cnt_w = np.zeros((ncores, nw), dtype=np.int64)
    for c in range(ncores):
        eids = np.nonzero(core_of == c)[0]
        dloc = (dst[eids] - c * nloc).astype(np.int64)
        order = np.argsort(dloc, kind="stable")
        eids = eids[order]
        dloc = dloc[order]
        w_of = dloc // P
        cnt_w[c] = np.bincount(w_of, minlength=nw)
        per_core.append((eids, dloc, w_of))

    # per-window block counts, equalized across cores; total padded to x16
    # (16 blocks = one 4-group DMA chunk)
    bws = np.maximum(1, (cnt_w.max(axis=0) + P - 1) // P)  # [nw]
    bws[-1] += (-int(bws.sum())) % 16
    btot = int(bws.sum())
    epad = btot * P

    block_window = np.repeat(np.arange(nw), bws)
    boff = np.concatenate([[0], np.cumsum(bws)[:-1]])  # first block of window

    in_maps = []
    for c in range(ncores):
        eids, dloc, w_of = per_core[c]
        # slot index for each edge: window base + position within window
        win_start = np.concatenate([[0], np.cumsum(cnt_w[c])[:-1]])
        pos_in_w = np.arange(len(eids)) - win_start[w_of]
        slot = boff[w_of] * P + pos_in_w  # [e_c]

        f8 = ml_dtypes.float8_e4m3

        xcat = np.zeros((epad, 2 * DIN), dtype=np.float32)
        xcat[slot, :DIN] = x[src[eids]]
        xcat[slot, DIN:] = x[dst[eids]]
        # DoubleRow layout: feature f = t*64 + p -> [p, t, e]
        xcat8 = np.ascontiguousarray(
            xcat.T.reshape(2, DIN, epad).transpose(1, 0, 2)
        ).astype(f8).reshape(DIN, 2 * epad)

        rbf = np.zeros((epad, NB), dtype=np.float32)
        rbf[slot] = rbf_all[eids]
        rbf8 = np.ascontiguousarray(
            rbf.T.reshape(2, NB // 2, epad).transpose(1, 0, 2)
        ).astype(f8).reshape(NB // 2, 2 * epad)

        dwrel = np.full(epad, 999.0, dtype=np.float32)
        dwrel[slot] = (dloc - w_of * P).astype(np.float32)
        dwrelT = np.ascontiguousarray(
            dwrel.reshape(btot, P).T).astype(bf16)  # [128, btot]

        cnt_n = np.zeros(npad, dtype=np.float32)
        cnt_n[:nloc] = np.bincount(dloc, minlength=nloc).astype(np.float32)
        inv = 1.0 / np.maximum(cnt_n, 1.0)
        has = (cnt_n > 0).astype(np.float32)
        fmul = np.broadcast_to(inv[None, :], (DOUT, npad)).copy()
        fadd = (np.asarray(mb2, np.float32)[:, None] * has[None, :]).copy()

        xt_loc = np.zeros((DIN, npad), dtype=np.float32)
        xt_loc[:, :nloc] = x[c * nloc:(c + 1) * nloc].T

        mw1_sd = np.asarray(mw1, np.float32)[:2 * DIN]     # [128, 128]
        mw18 = np.ascontiguousarray(
            mw1_sd.reshape(2, DIN, 2 * DOUT).transpose(1, 0, 2)
        ).astype(f8).reshape(DIN, 2 * (2 * DOUT))
        mw1_r = np.asarray(mw1, np.float32)[2 * DIN:]      # [16, 128]
        mw1r8 = np.ascontiguousarray(
            mw1_r.reshape(2, NB // 2, 2 * DOUT).transpose(1, 0, 2)
        ).astype(f8).reshape(NB // 2, 2 * (2 * DOUT))

        m = {
            "xcat8": xcat8,
            "rbf8": rbf8,
            "dwrelT": dwrelT,
            "xTloc": xt_loc.astype(bf16),
            "fmul": fmul,
            "fadd": fadd,
            "mw18": mw18,
            "mw1r8": mw1r8,
            "mb1": np.asarray(mb1, np.float32).reshape(2 * DOUT, 1).copy(),
            "mw2": np.asarray(mw2, np.float32).astype(bf16),
            # upd layout is [agg; x] -> swap uw1 row blocks to match
            "uw1": np.concatenate([np.asarray(uw1, np.float32)[DIN:],
                                   np.asarray(uw1, np.float32)[:DIN]],
                                  axis=0).astype(bf16),
            "ub1": np.asarray(ub1, np.float32).reshape(DOUT, 1).copy(),
            "uw2": np.asarray(uw2, np.float32).astype(bf16),
            "ub2": np.asarray(ub2, np.float32).reshape(DOUT, 1).copy(),
            "lng": np.broadcast_to(np.asarray(ln_g, np.float32)[None, :],
                                   (P, DOUT)).copy(),
            "lnb": np.broadcast_to(np.asarray(ln_b, np.float32)[None, :],
                                   (P, DOUT)).copy(),
            "iota4": np.tile(np.arange(P, dtype=np.float32).astype(bf16),
                             (P, 4)).copy(),
            "identf": np.eye(P, dtype=np.float32),
        }
        in_maps.append(m)

    struct = dict(n=n, nloc=nloc, nw=nw, npad=npad, btot=btot, epad=epad,
                  bws=tuple(int(v) for v in bws),
                  block_window=tuple(int(v) for v in block_window))
    return struct, in_maps


# ---------------------------------------------------------------------------
# Device program
# ---------------------------------------------------------------------------

def _build_program(struct):
    import concourse.bass as bass
    import concourse.mybir as mybir
    import concourse.tile as tile
    from concourse import bacc

    f32 = mybir.dt.float32
    bf = mybir.dt.bfloat16
    f8 = mybir.dt.float8e4
    DR = mybir.MatmulPerfMode.DoubleRow
    n, nloc, nw, npad = (struct["n"], struct["nloc"], struct["nw"],
                         struct["npad"])
    btot, epad = struct["btot"], struct["epad"]
    block_window = struct["block_window"]
    ngrp = btot // 4

    # first/last block of each window
    wfirst = {}
    wlast = {}
    for g, w in enumerate(block_window):
        wfirst.setdefault(w, g)
        wlast[w] = g

    nc = bacc.Bacc("TRN2", target_bir_lowering=False, debug=False,
                   enable_asserts=False, num_devices=NCORES)

    xcat8_d = nc.dram_tensor("xcat8", [DIN, 2 * epad], f8,
                             kind="ExternalInput")
    rbf8_d = nc.dram_tensor("rbf8", [NB // 2, 2 * epad], f8,
                            kind="ExternalInput")
    dwrelT_d = nc.dram_tensor("dwrelT", [P, btot], bf, kind="ExternalInput")
    xTloc_d = nc.dram_tensor("xTloc", [DIN, npad], bf, kind="ExternalInput")
    fmul_d = nc.dram_tensor("fmul", [DOUT, npad], f32, kind="ExternalInput")
    fadd_d = nc.dram_tensor("fadd", [DOUT, npad], f32, kind="ExternalInput")
    mw18_d = nc.dram_tensor("mw18", [DIN, 2 * (2 * DOUT)], f8,
                            kind="ExternalInput")
    mw1r8_d = nc.dram_tensor("mw1r8", [NB // 2, 2 * (2 * DOUT)], f8,
                             kind="ExternalInput")
    mb1_d = nc.dram_tensor("mb1", [2 * DOUT, 1], f32, kind="ExternalInput")
    mw2_d = nc.dram_tensor("mw2", [2 * DOUT, DOUT], bf, kind="ExternalInput")
    uw1_d = nc.dram_tensor("uw1", [DIN + DOUT, DOUT], bf,
                           kind="ExternalInput")
    ub1_d = nc.dram_tensor("ub1", [DOUT, 1], f32, kind="ExternalInput")
    uw2_d = nc.dram_tensor("uw2", [DOUT, DOUT], bf, kind="ExternalInput")
    ub2_d = nc.dram_tensor("ub2", [DOUT, 1], f32, kind="ExternalInput")
    lng_d = nc.dram_tensor("lng", [P, DOUT], f32, kind="ExternalInput")
    lnb_d = nc.dram_tensor("lnb", [P, DOUT], f32, kind="ExternalInput")
    iota4_d = nc.dram_tensor("iota4", [P, 4 * P], bf, kind="ExternalInput")
    identf_d = nc.dram_tensor("identf", [P, P], f32, kind="ExternalInput")
    out_d = nc.dram_tensor("out", [npad, DOUT], f32, kind="ExternalOutput")

    AX = mybir.AxisListType
    OP = mybir.AluOpType
    ACT = mybir.ActivationFunctionType

    with tile.TileContext(nc) as tc:
        with (
            tc.tile_pool(name="const", bufs=1) as cpool,
            tc.tile_pool(name="gath", bufs=5) as gpool,
            tc.tile_pool(name="work", bufs=4) as wpool,
            tc.tile_pool(name="oh", bufs=3) as opool,
            tc.tile_pool(name="pt", bufs=1, space="PSUM") as pt_pool,
            tc.tile_pool(name="ph", bufs=3, space="PSUM") as ph_pool,
            tc.tile_pool(name="pm", bufs=2, space="PSUM") as pm_pool,
            tc.tile_pool(name="pa", bufs=2, space="PSUM") as pa_pool,
        ):
            def cload(dram, shape, dtype=f32):
                t = cpool.tile(shape, dtype, name=dram.name + "_t")
                nc.sync.dma_start(out=t[:], in_=dram[:])
                return t

            mw18_t = cload(mw18_d, [DIN, 2 * (2 * DOUT)], f8)
            mw1r8_t = cload(mw1r8_d, [NB // 2, 2 * (2 * DOUT)], f8)
            mb1_t = cload(mb1_d, [2 * DOUT, 1])
            mw2_t = cload(mw2_d, [2 * DOUT, DOUT], bf)
            uw1_t = cload(uw1_d, [DIN + DOUT, DOUT], bf)
            ub1_t = cload(ub1_d, [DOUT, 1])
            uw2_t = cload(uw2_d, [DOUT, DOUT], bf)
            ub2_t = cload(ub2_d, [DOUT, 1])
            lng_t = cload(lng_d, [P, DOUT])
            lnb_t = cload(lnb_d, [P, DOUT])
            iota4_t = cload(iota4_d, [P, 4 * P], bf)
            identf_t = cload(identf_d, [P, P])
            dwrelT_t = cload(dwrelT_d, [P, btot], bf)
            fmul_t = cload(fmul_d, [DOUT, npad])
            fadd_t = cload(fadd_d, [DOUT, npad])

            eps_t = cpool.tile([P, 1], f32, name="eps_t")
            nc.vector.memset(eps_t[:], 1e-5)

            # combined update-MLP input: rows 0:64 = aggT, rows 64:128 = xT
            upd_t = cpool.tile([P, npad], bf, name="upd_t")
            nc.sync.dma_start(out=upd_t[DOUT:P, :], in_=xTloc_d[:])

            # pipeline state
            st = {}     # k -> dict(ph, hT, pm, msg, oh4)
            chunks = {}  # c -> (xc4, rb4)
            pa_cur = {}
            flushed_w = [-1]
            next_u = [0]
            nug = (npad + 511) // 512
            CH = 4 * 4 * P  # edges per DMA chunk (4 groups)

            mw18_v = mw18_t[:].rearrange("p (t m) -> p t m", t=2)
            mw1r8_v = mw1r8_t[:].rearrange("p (t m) -> p t m", t=2)
            xcat8_v = xcat8_d[:].rearrange("p (t e) -> p t e", t=2)
            rbf8_v = rbf8_d[:].rearrange("p (t e) -> p t e", t=2)

            def stage_dma(c):
                e0 = c * CH
                xc4 = gpool.tile([DIN, 2 * CH], f8, tag="xc", name=f"xc_{c}")
                nc.sync.dma_start(
                    out=xc4[:].rearrange("p (t e) -> p t e", t=2),
                    in_=xcat8_v[:, :, e0:e0 + CH])
                rb4 = gpool.tile([NB // 2, 2 * CH], f8, tag="rb",
                                 name=f"rb_{c}")
                nc.sync.dma_start(
                    out=rb4[:].rearrange("p (t e) -> p t e", t=2),
                    in_=rbf8_v[:, :, e0:e0 + CH])
                chunks[c] = (xc4, rb4)

            def stage_l1(k):
                s = st[k] = {}
                xc4, rb4 = chunks[k // 4]
                q = (k % 4) * 4 * P
                xcv = xc4[:].rearrange("p (t e) -> p t e", t=2)
                rbv = rb4[:].rearrange("p (t e) -> p t e", t=2)
                ph = s["ph"] = ph_pool.tile([P, 4 * P], f32, tag="ph",
                                            name=f"ph_{k}")
                nc.tensor.matmul(ph[:], mw18_v, xcv[:, :, q:q + 4 * P],
                                 start=True, stop=False, perf_mode=DR)
                nc.tensor.matmul(ph[:], mw1r8_v, rbv[:, :, q:q + 4 * P],
                                 start=False, stop=True, perf_mode=DR)
                if k % 4 == 3:
                    del chunks[k // 4]
                hT = s["hT"] = wpool.tile([P, 4 * P], bf, tag="hT",
                                          name=f"hT_{k}")
                nc.scalar.activation(out=hT[:], in_=ph[:],
                                     func=ACT.Silu, bias=mb1_t[:, 0:1])
                # batched one-hot for the 4 blocks of this group
                oh4 = s["oh4"] = opool.tile([P, 4 * P], bf, tag="oh4",
                                            name=f"oh4_{k}")
                o3 = oh4[:].rearrange("p (j n) -> p j n", n=P)
                i3 = iota4_t[:].rearrange("p (j n) -> p j n", n=P)
                nc.vector.tensor_tensor(
                    out=o3, in0=i3,
                    in1=dwrelT_t[:, 4 * k:4 * k + 4, None]
                        .to_broadcast([P, 4, P]),
                    op=OP.is_equal)

            def stage_l2(k):
                s = st[k]
                hT = s["hT"]
                pm = s["pm"] = pm_pool.tile([P, 4 * DOUT], f32, tag="pm",
                                            name=f"pm_{k}")
                for j in range(4):
                    nc.tensor.matmul(pm[:, j * DOUT:(j + 1) * DOUT],
                                     hT[:, j * P:(j + 1) * P],
                                     mw2_t[:], start=True, stop=True)
                msg = s["msg"] = wpool.tile([P, 4 * DOUT], bf, tag="msg",
                                            name=f"msg_{k}")
                nc.scalar.copy(out=msg[:], in_=pm[:])

            def stage_scatter(k):
                s = st[k]
                msg, oh4 = s["msg"], s["oh4"]
                for j in range(4):
                    g = 4 * k + j
                    w = block_window[g]
                    if g == wfirst[w]:
                        pa_cur[w] = pa_pool.tile([DOUT, P], f32, tag="pa",
                                                 name=f"pa_w{w}")
                    nc.tensor.matmul(
                        pa_cur[w][:],
                        msg[:, j * DOUT:(j + 1) * DOUT],
                        oh4[:, j * P:(j + 1) * P],
                        start=(g == wfirst[w]),
                        stop=(g == wlast[w]),
                        skip_group_check=True)
                    if g != wlast[w]:
                        continue
                    wc = slice(w * P, (w + 1) * P)
                    nc.vector.tensor_tensor(
                        out=upd_t[0:DOUT, wc], in0=pa_cur[w][:],
                        in1=fmul_t[:, wc], op=OP.mult)
                    nc.vector.tensor_tensor(
                        out=upd_t[0:DOUT, wc], in0=upd_t[0:DOUT, wc],
                        in1=fadd_t[:, wc], op=OP.add)
                    del pa_cur[w]
                    flushed_w[0] = w
                del st[k]

            def stage_update(m):
                u0 = m * 512
                cw = min(512, npad - u0)
                nj = cw // P
                pu = ph_pool.tile([P, 512], f32, tag="ph", name=f"pu_{m}")
                nc.tensor.matmul(pu[0:DOUT, 0:cw], uw1_t[:],
                                 upd_t[:, u0:u0 + cw], start=True, stop=True)
                uh_sb = wpool.tile([DOUT, 512], bf, tag="uh", name=f"uh_{m}")
                nc.scalar.activation(out=uh_sb[:, 0:cw],
                                     in_=pu[0:DOUT, 0:cw],
                                     func=ACT.Silu, bias=ub1_t[:, 0:1])
                pz = pt_pool.tile([P, 512], f32, tag="pz", name=f"pz_{m}")
                nc.tensor.matmul(pz[0:DOUT, 0:cw], uw2_t[:], uh_sb[:, 0:cw],
                                 start=True, stop=True)
                zT_sb = wpool.tile([DOUT, 512], f32, tag="zT", name=f"zT_{m}")
                nc.scalar.activation(out=zT_sb[:, 0:cw], in_=pz[0:DOUT, 0:cw],
                                     func=ACT.Identity, bias=ub2_t[:, 0:1])

                pz2 = pm_pool.tile([P, 4 * DOUT], f32, tag="pm",
                                   name=f"pz2_{m}")
                for j in range(nj):
                    nc.tensor.transpose(
                        out=pz2[:, j * DOUT:(j + 1) * DOUT],
                        in_=zT_sb[:, j * P:(j + 1) * P],
                        identity=identf_t[0:DOUT, 0:DOUT])
                # LayerNorm on [128, nj, 64] (free-axis per-node)
                zc = wpool.tile([P, 4 * DOUT], f32, tag="zc", name=f"zc_{m}")
                red = wpool.tile([P, 4], f32, tag="red", name=f"red_{m}")
                red2 = wpool.tile([P, 4], f32, tag="red2", name=f"red2_{m}")
                z3 = pz2[:, 0:nj * DOUT].rearrange("p (j d) -> p j d", d=DOUT)
                nc.vector.tensor_reduce(out=red[:, 0:nj], in_=z3, axis=AX.X,
                                        op=OP.add)
                nc.vector.tensor_scalar_mul(red[:, 0:nj], red[:, 0:nj],
                                            -1.0 / DOUT)
                zc3 = zc[:, 0:nj * DOUT].rearrange("p (j d) -> p j d", d=DOUT)
                nc.vector.tensor_tensor(
                    out=zc3, in0=z3,
                    in1=red[:, 0:nj, None].to_broadcast([P, nj, DOUT]),
                    op=OP.add)
                sq = wpool.tile([P, 4 * DOUT], f32, tag="sq", name=f"sq_{m}")
                sq3 = sq[:, 0:nj * DOUT].rearrange("p (j d) -> p j d", d=DOUT)
                nc.vector.tensor_tensor(out=sq3, in0=zc3, in1=zc3, op=OP.mult)
                nc.vector.tensor_reduce(out=red2[:, 0:nj], in_=sq3, axis=AX.X,
                                        op=OP.add)
                sd = wpool.tile([P, 4], f32, tag="sd", name=f"sd_{m}")
                nc.scalar.activation(out=sd[:, 0:nj], in_=red2[:, 0:nj],
                                     func=ACT.Sqrt, scale=1.0 / DOUT,
                                     bias=eps_t[:, 0:1])
                rs = wpool.tile([P, 4], f32, tag="rs", name=f"rs_{m}")
                nc.vector.reciprocal(out=rs[:, 0:nj], in_=sd[:, 0:nj])
                zn = wpool.tile([P, 4 * DOUT], f32, tag="zn", name=f"zn_{m}")
                zn3 = zn[:, 0:nj * DOUT].rearrange("p (j d) -> p j d", d=DOUT)
                nc.vector.tensor_tensor(
                    out=zn3, in0=zc3,
                    in1=rs[:, 0:nj, None].to_broadcast([P, nj, DOUT]),
                    op=OP.mult)
                for j in range(nj):
                    js = slice(j * DOUT, (j + 1) * DOUT)
                    nc.vector.tensor_tensor(out=zn[:, js], in0=zn[:, js],
                                            in1=lng_t[:], op=OP.mult)
                    nc.vector.tensor_tensor(out=zn[:, js], in0=zn[:, js],
                                            in1=lnb_t[:], op=OP.add)
                    r0 = u0 + j * P
                    nc.sync.dma_start(out=out_d[r0:r0 + P, :],
                                      in_=zn[:, js])

            nch = ngrp // 4
            stage_dma(0)
            for k in range(ngrp + 2):
                if k < ngrp:
                    if k % 4 == 0 and k // 4 + 1 < nch:
                        stage_dma(k // 4 + 1)
                    stage_l1(k)
                if 1 <= k <= ngrp:
                    stage_l2(k - 1)
                if k >= 2:
                    stage_scatter(k - 2)
                    while (next_u[0] < nug
                           and flushed_w[0] >= min(4 * next_u[0] + 3,
                                                   nw - 1)):
                        stage_update(next_u[0])
                        next_u[0] += 1
            while next_u[0] < nug:
                stage_update(next_u[0])
                next_u[0] += 1

    nc.compile()
    return nc


# ---------------------------------------------------------------------------
# Entry point
# ---------------------------------------------------------------------------

last_results = None


def kernel(x, edge_index, edge_vec, edge_len,
           mw1, mb1, mw2, mb2, uw1, ub1, uw2, ub2, ln_g, ln_b):
    global last_results
    import os
    from concourse.bass_utils import run_bass_kernel_spmd

    struct, in_maps = _build_host_data(
        x, edge_index, edge_len, mw1, mb1, mw2, mb2,
        uw1, ub1, uw2, ub2, ln_g, ln_b)

    key = (struct["n"], struct["btot"], struct["bws"])
    if key not in _prog_cache:
        _prog_cache[key] = _build_program(struct)
    nc = _prog_cache[key]

    kw = {}
    if os.environ.get("K_TRACE", ""):
        import profile_shim
        profile_shim.install()
        kw = dict(trace=True, trace_cores=list(range(NCORES)),
                  tmpdir="/tmp/ntff_out")
    res = run_bass_kernel_spmd(nc, in_maps, core_ids=list(range(NCORES)), **kw)
    last_results = res
    nloc = struct["nloc"]
    out = np.concatenate([res.results[c]["out"][:nloc] for c in range(NCORES)],
                         axis=0)
    return out.astype(np.float32)
